# revision 67
# baseline (speedup 1.0000x reference)
"""JacobiGNN Trainium2 kernel: out = log_softmax(U @ (H * (U^T z)), axis=1).

Per-core sharding (core c of 8):
  GEMM1 (row-slab, no z gather): z_c = MLP(x rows) stays local;
    P^T[16,8192] += z_k^T[16,128] @ U[rows_c,:][k][128,8192]   (8 chunks)
    ReduceScatter over spectral -> G_c^T[16,1024].
  GEMM2 (col-slab): out^T[16,8192] += y_sk^T[16,128] @ U^T[cols_c][sk]
    ReduceScatter over nodes -> out rows for core c.

All GEMMs stream U/U^T from DRAM as the bf16 moving operand against tiny
16-wide stationary z/y chunks (1 cycle/row, f32 PSUM accumulation), so the
kernel is HBM-bound at ~32MB/core. out^T/P^T accumulate packed into 4 PSUM
partition-groups (32q offsets). The spectral ReduceScatter hides under the
U^T stream; a dummy sync-engine DMA gates the U^T enqueue so the eviction
descriptors are not buried behind 16MB of queued stream traffic.
"""

import os
import sys

import numpy as np
import ml_dtypes

for _p in ("/opt/trn_rl_repo", "/root/.axon_site/_ro/trn_rl_repo"):
    if os.path.isdir(_p) and _p not in sys.path:
        sys.path.insert(0, _p)

import concourse.bacc as bacc
import concourse.bass as bass  # noqa: F401
import concourse.mybir as mybir
import concourse.tile as tile
from concourse.bass_utils import run_bass_kernel_spmd

F32 = mybir.dt.float32
BF16 = mybir.dt.bfloat16
NPBF16 = ml_dtypes.bfloat16

N, F_IN, HID, C, K = 8192, 512, 64, 16, 10
BASE_ALPHA = 0.5
JA, JB, JL, JR = 1.0, 1.0, -1.0, 1.0
NCORES = 8
SH = N // NCORES          # local spectral columns / node rows (1024)
SK = SH // 128            # local 128-chunks (8)
QN = 4                    # PSUM partition-groups for [16, 8192] packing
QCOLS = N // QN           # cols per group (2048)

_CACHE = {}


def _node_perm():
    """Pack-column -> node for ut_pm. Halves h = pack//4096 are separate
    ReduceScatters; within half h, shard j's slot i maps to outT column
    m = h*512 + i of core j, which after the PE transpose must hold node
    (m%128)*SK + m//128 so the output DMA writes contiguous runs."""
    p = np.arange(N)
    h = p // 4096
    s = p % 4096
    j = s // 512
    m = h * 512 + s % 512
    return j * SH + (m % 128) * SK + m // 128


def _spectral_perm():
    """Pack-column -> spectral for u_row: PSUM quarter q, shard-slot (j, i)
    holds spectral j*1024 + q*256 + i, so each quarter evicts directly as
    one ReduceScatter input and G arrives in 4 pipelined pieces."""
    p = np.arange(N)
    q = p // QCOLS
    j = (p % QCOLS) // 256
    i = p % 256
    return j * SH + q * 256 + i


def _jacobi_coef_rows(temp):
    """Host-precomputed per-channel coefficient rows, [30*C] packed."""
    a, b, l, r = JA, JB, JL, JR
    alphas = (BASE_ALPHA * np.tanh(np.asarray(temp, np.float64)))  # [C, K+1]
    rows = [alphas[:, 0]]
    coef1 = (a - b) / 2 - (a + b + 2) / 2 * (l + r) / (r - l)
    coef2 = (a + b + 2) / (r - l)
    rows.append(coef1 * alphas[:, 1])   # c1_0
    rows.append(coef2 * alphas[:, 1])   # c1_1
    for L in range(2, K + 1):
        coef_l = 2 * L * (L + a + b) * (2 * L - 2 + a + b)
        c_lm1_1 = (2 * L + a + b - 1) * (2 * L + a + b) * (2 * L + a + b - 2)
        c_lm1_2 = (2 * L + a + b - 1) * (a ** 2 - b ** 2)
        c_lm2 = 2 * (L - 1 + a) * (L - 1 + b) * (2 * L + a + b)
        tmp1 = alphas[:, L - 1] * (c_lm1_1 / coef_l)
        tmp2 = alphas[:, L - 1] * (c_lm1_2 / coef_l)
        tmp3 = alphas[:, L - 1] * alphas[:, L - 2] * (c_lm2 / coef_l)
        rows.append(tmp1 * (2 / (r - l)))                    # t1
        rows.append(tmp1 * ((r + l) / (r - l)) + tmp2)       # t2
        rows.append(tmp3)                                    # t3
    packed = np.concatenate(rows).astype(np.float32).reshape(1, 30 * C)
    return np.ascontiguousarray(np.repeat(packed, 128, axis=0))


def _bc(ap, shape, axis=1):
    while ap.ndim < len(shape):
        ap = ap.unsqueeze(axis)
    return ap.broadcast_to(shape)


def _build():
    nc = bacc.Bacc("TRN2", target_bir_lowering=False, debug=False)

    u_row = nc.dram_tensor("u_row", [SH, N], BF16, kind="ExternalInput")
    ut_pm = nc.dram_tensor("ut_pm", [SH, N], BF16, kind="ExternalInput")
    x_sh = nc.dram_tensor("x_shard", [F_IN, SH], BF16, kind="ExternalInput")
    e_sh = nc.dram_tensor("e_shard", [128, SK], F32, kind="ExternalInput")
    w1r = nc.dram_tensor("w1r", [128, 4 * HID], BF16, kind="ExternalInput")
    w2d = nc.dram_tensor("w2d", [HID, C], BF16, kind="ExternalInput")
    b1c = nc.dram_tensor("b1c", [HID, 1], F32, kind="ExternalInput")
    b2c = nc.dram_tensor("b2c", [C, 1], F32, kind="ExternalInput")
    jcd = nc.dram_tensor("jcd", [128, 30 * C], F32, kind="ExternalInput")
    id16d = nc.dram_tensor("id16d", [C, C], F32, kind="ExternalInput")
    out_sh = nc.dram_tensor("out_shard", [SH, C], F32, kind="ExternalOutput")

    rg = [list(range(NCORES))]

    with nc.allow_low_precision(reason="bf16 matmul path"), \
         tile.TileContext(nc) as tc:
        with (
            tc.tile_pool(name="dram", bufs=1, space="DRAM") as dram,
            tc.tile_pool(name="consts", bufs=1) as cp,
            tc.tile_pool(name="persist", bufs=1) as pp,
            tc.tile_pool(name="usb", bufs=16) as up,
            tc.tile_pool(name="small", bufs=4) as sp,
        ):
            rs_g_in = dram.tile([NCORES * C, SH], BF16)
            rs_g_out = dram.tile([C, SH], BF16)
            rs_o_in = [dram.tile([NCORES * C, 512], BF16, name=f"rs_o_in{h}")
                       for h in range(2)]
            rs_o_out = [dram.tile([C, 512], BF16, name=f"rs_o_out{h}")
                        for h in range(2)]
            gate = pp.tile([C, 64], BF16)

            id16 = cp.tile_from(id16d[:])
            jc = cp.tile_from(jcd[:])
            w1 = cp.tile_from(w1r[:])
            w2 = cp.tile_from(w2d[:])
            b1 = cp.tile_from(b1c[:])
            b2 = cp.tile_from(b2c[:])
            e_col = cp.tile_from(e_sh[:])

            zme = pp.tile([128, SK, C], BF16)
            hacc = pp.tile([128, SK, C], F32)
            xs_a = pp.tile([128, SK, C], F32)
            xs_b = pp.tile([128, SK, C], F32)
            htmp = pp.tile([128, SK, C], F32)
            htmp2 = pp.tile([128, SK, C], F32)
            gsbT = pp.tile([C, SH], F32)
            gcol = pp.tile([128, SK, C], F32)
            ybf = pp.tile([128, SK, C], BF16)
            accg = pp.tile([128, QCOLS], BF16)
            acco = pp.tile([128, QCOLS], BF16)
            outT = pp.tile([C, SH], F32)
            smin = pp.tile([128, SK, C], F32)
            smout = pp.tile([128, SK, C], F32)

            # ================= phase 0: MLP head -> z (local only) =======
            with tc.tile_pool(name="mlp", bufs=1) as mp, \
                 tc.tile_pool(name="ppre", bufs=1, space="PSUM") as ppre:
                xT = mp.tile([128, 4, SH], BF16)
                h_sb = mp.tile([HID, SH], BF16)
                zT = mp.tile([C, SH], F32)
                nc.scalar.dma_start(
                    out=xT[:], in_=x_sh[:].rearrange("(a p) r -> p a r", p=128))
                ph = ppre.tile([HID, SH], F32, tag="ph")
                for half in range(2):
                    for fb in range(4):
                        nc.tensor.matmul(
                            ph[:, half * 512:(half + 1) * 512],
                            lhsT=w1[:, fb * HID:(fb + 1) * HID],
                            rhs=xT[:, fb, half * 512:(half + 1) * 512],
                            start=(fb == 0), stop=(fb == 3),
                        )
                nc.scalar.activation(h_sb[:], ph[:], mybir.ActivationFunctionType.Relu,
                                     bias=b1[:, 0:1], scale=1.0)
                pz = ppre.tile([C, SH], F32, tag="pz")
                for half in range(2):
                    nc.tensor.matmul(
                        pz[:, half * 512:(half + 1) * 512],
                        lhsT=w2[:], rhs=h_sb[:, half * 512:(half + 1) * 512],
                        start=True, stop=True,
                    )
                nc.vector.tensor_scalar_add(zT[:], pz[:], b2[:, 0:1])
                # preload Exp/Ln activation tables so the softmax tail does
                # not pay the 1.3us table load
                dex = sp.tile([C, 1], F32, tag="dex")
                nc.scalar.activation(dex[:], b2[:], mybir.ActivationFunctionType.Exp)
                nc.scalar.activation(dex[:], dex[:], mybir.ActivationFunctionType.Ln)
                for rc in range(SK):
                    pzt = ppre.tile([128, C], F32, tag="pzt", bufs=3)
                    nc.tensor.transpose(pzt[:], zT[:, rc * 128:(rc + 1) * 128],
                                        id16[0:C, 0:C])
                    nc.scalar.copy(zme[:, rc, :], pzt[:])

            # utsb opens after the MLP pool closed so it reuses that space
            utp_cm = tc.tile_pool(name="utsb", bufs=28)
            utp = utp_cm.__enter__()

            # ================= Jacobi filter H on DVE ====================
            ev = _bc(e_col[:], (128, SK, C), axis=2)

            def jrow(i):
                return _bc(jc[:, i * C:(i + 1) * C], (128, SK, C))

            nc.vector.tensor_copy(xs_a[:], jrow(0))                       # xs_m2
            nc.vector.tensor_mul(htmp[:], xs_a[:], ev)
            nc.vector.tensor_mul(htmp[:], htmp[:], jrow(2))
            nc.vector.tensor_add(xs_b[:], htmp[:], jrow(1))               # xs_m1
            nc.vector.tensor_add(hacc[:], xs_a[:], xs_b[:])
            xm2, xm1 = xs_a, xs_b
            for L in range(2, K + 1):
                r0 = 3 + 3 * (L - 2)
                nc.vector.tensor_mul(htmp[:], xm1[:], ev)
                nc.vector.tensor_mul(htmp[:], htmp[:], jrow(r0))
                nc.vector.tensor_mul(htmp2[:], xm1[:], jrow(r0 + 1))
                nc.vector.tensor_sub(htmp[:], htmp[:], htmp2[:])
                nc.vector.tensor_mul(htmp2[:], xm2[:], jrow(r0 + 2))
                nc.vector.tensor_sub(xm2[:], htmp[:], htmp2[:])           # nx
                nc.vector.tensor_add(hacc[:], hacc[:], xm2[:])
                xm2, xm1 = xm1, xm2

            # ============ GEMM1: P^T = z^T @ U[rows], then RS-G ==========
            with tc.tile_pool(name="pg", bufs=1, space="PSUM") as pg:
                pacc_g = pg.tile([128, QCOLS], F32, tag="paccg")
                for k in range(SK):
                    for q in range(QN):
                        upc = up.tile([128, QCOLS], BF16, tag="u")
                        nc.sync.dma_start(
                            out=upc[:],
                            in_=u_row[k * 128:(k + 1) * 128,
                                      q * QCOLS:(q + 1) * QCOLS])
                        for off in range(4):
                            nc.tensor.matmul(
                                pacc_g[32 * q:32 * q + C, off * 512:(off + 1) * 512],
                                lhsT=zme[:, k, :], rhs=upc[:, off * 512:(off + 1) * 512],
                                start=(k == 0), stop=(k == SK - 1),
                                skip_group_check=True,
                                tile_position=(0, 32 * q),
                            )
                nc.vector.tensor_copy(accg[0:64, :], pacc_g[0:64, :])
                nc.scalar.copy(accg[64:128, :], pacc_g[64:128, :])
            for j in range(NCORES):
                eng = nc.gpsimd if j % 2 == 0 else nc.scalar
                eng.dma_start(
                    out=rs_g_in[j * C:(j + 1) * C, :],
                    in_=accg[32 * (j // 2):32 * (j // 2) + C,
                             (j % 2) * SH:(j % 2 + 1) * SH])
            # gate: the 16MB U^T stream may enqueue only after the RS-G
            # eviction DMAs have landed (keeps them out of queue burial).
            # Must be emitted BEFORE the collective: Tile models the
            # collective as writing rs_g_in, which would serialize the gate
            # (and the whole U^T stream) behind the full RS.
            nc.sync.dma_start(out=gate[:], in_=rs_g_in[0:C, 0:64])
            nc.gpsimd.collective_compute(
                "ReduceScatter", mybir.AluOpType.add, replica_groups=rg,
                ins=[rs_g_in.opt()], outs=[rs_g_out.opt()],
            )

            # y = H * G (transpose G^T back to node-partition layout)
            nc.gpsimd.dma_start(out=gsbT[:], in_=rs_g_out[:])  # bf16 -> f32 cast
            with tc.tile_pool(name="pt", bufs=1, space="PSUM") as pt:
                for sb in range(SK):
                    ptile = pt.tile([128, C], F32, tag="pt", bufs=2)
                    nc.tensor.transpose(ptile[:], gsbT[:, sb * 128:(sb + 1) * 128],
                                        id16[0:C, 0:C])
                    nc.vector.tensor_copy(gcol[:, sb, :], ptile[:])
            nc.vector.tensor_mul(ybf[:], gcol[:], hacc[:])

            # ==== GEMM2: out^T = y^T @ U^T[cols], 2 pipelined RS-out =====
            # q-outer so node-half h completes after quarters 2h, 2h+1; its
            # ReduceScatter + softmax + output DMA hide under half h+1
            sums8 = sp.tile([128, SK], F32, tag="sums")
            lns8 = sp.tile([128, SK], F32, tag="lns")
            et8 = sp.tile([128, SK, C], F32, tag="et")
            with tc.tile_pool(name="pm", bufs=1, space="PSUM") as pm:
                pacc_o = pm.tile([128, QCOLS], F32, tag="pacco")
                for h in range(2):
                    for q in (2 * h, 2 * h + 1):
                        for sk in range(SK):
                            utt = utp.tile([128, QCOLS], BF16, tag="ut")
                            nc.sync.dma_start(
                                out=utt[:],
                                in_=ut_pm[sk * 128:(sk + 1) * 128,
                                          q * QCOLS:(q + 1) * QCOLS])
                            for off in range(4):
                                nc.tensor.matmul(
                                    pacc_o[32 * q:32 * q + C,
                                           off * 512:(off + 1) * 512],
                                    lhsT=ybf[:, sk, :],
                                    rhs=utt[:, off * 512:(off + 1) * 512],
                                    start=(sk == 0), stop=(sk == SK - 1),
                                    skip_group_check=True,
                                    tile_position=(0, 32 * q),
                                )
                    ceng = nc.vector.tensor_copy if h == 0 else nc.scalar.copy
                    ceng(acco[64 * h:64 * h + 48, :], pacc_o[64 * h:64 * h + 48, :])
                    for j in range(NCORES):
                        eng = nc.gpsimd if j % 2 == 0 else nc.scalar
                        eng.dma_start(
                            out=rs_o_in[h][j * C:(j + 1) * C, :],
                            in_=acco[32 * (2 * h + j // 4):32 * (2 * h + j // 4) + C,
                                     (j % 4) * 512:(j % 4 + 1) * 512])
                    nc.gpsimd.collective_compute(
                        "ReduceScatter", mybir.AluOpType.add, replica_groups=rg,
                        ins=[rs_o_in[h].opt()], outs=[rs_o_out[h].opt()],
                    )
                    # local log_softmax + output for node half h
                    nc.gpsimd.dma_start(out=outT[:, h * 512:(h + 1) * 512],
                                        in_=rs_o_out[h][:])  # bf16 -> f32 cast
                    for sb in range(4 * h, 4 * h + 4):
                        ptile = pm.tile([128, C], F32, tag="pt", bufs=2)
                        nc.tensor.transpose(ptile[:], outT[:, sb * 128:(sb + 1) * 128],
                                            id16[0:C, 0:C])
                        nc.scalar.copy(smin[:, sb, :], ptile[:])
                    hs = slice(4 * h, 4 * h + 4)
                    nc.scalar.activation(et8[:, hs, :], smin[:, hs, :],
                                         mybir.ActivationFunctionType.Exp)
                    nc.vector.tensor_reduce(out=sums8[:, hs], in_=et8[:, hs, :],
                                            op=mybir.AluOpType.add,
                                            axis=mybir.AxisListType.X)
                    nc.scalar.activation(lns8[:, hs], sums8[:, hs],
                                         mybir.ActivationFunctionType.Ln)
                    nc.vector.tensor_sub(smout[:, hs, :], smin[:, hs, :],
                                         _bc(lns8[:, hs], (128, 4, C), axis=2))
                    # in-shard node perm makes row r = p*SK + j, so each
                    # partition writes contiguous 256B runs
                    nc.scalar.dma_start(
                        out=out_sh[:].rearrange("(p j) c -> p j c", p=128)[:, hs, :],
                        in_=smout[:, hs, :])
            utp_cm.__exit__(None, None, None)

    nc.compile()
    return nc


def _prep_inputs(origin_e, U, x, W1, b1, W2, b2, temp):
    origin_e = np.asarray(origin_e, np.float32)
    U = np.asarray(U, np.float32)
    x = np.asarray(x, np.float32)
    W1 = np.asarray(W1, np.float32)
    b1 = np.asarray(b1, np.float32)
    W2 = np.asarray(W2, np.float32)
    b2 = np.asarray(b2, np.float32)

    jc = _jacobi_coef_rows(temp)
    id16 = np.eye(C, dtype=np.float32)
    w1r = np.ascontiguousarray(
        W1.reshape(4, 128, HID).transpose(1, 0, 2).reshape(128, 4 * HID)
    ).astype(NPBF16)
    UT = np.ascontiguousarray(U.T[:, _node_perm()])
    shared = {
        "w1r": w1r, "w2d": np.ascontiguousarray(W2).astype(NPBF16),
        "b1c": np.ascontiguousarray(b1.reshape(HID, 1)),
        "b2c": np.ascontiguousarray(b2.reshape(C, 1)),
        "jcd": jc, "id16d": id16,
    }
    in_maps = []
    for i in range(NCORES):
        m = dict(shared)
        m["u_row"] = U[i * SH:(i + 1) * SH, :].astype(NPBF16)
        m["ut_pm"] = UT[i * SH:(i + 1) * SH, :].astype(NPBF16)
        m["x_shard"] = np.ascontiguousarray(x[i * SH:(i + 1) * SH, :].T).astype(NPBF16)
        m["e_shard"] = np.ascontiguousarray(
            origin_e[i * SH:(i + 1) * SH].reshape(SK, 128).T)
        in_maps.append(m)
    return in_maps


def _get_program():
    if "nc" not in _CACHE:
        _CACHE["nc"] = _build()
    return _CACHE["nc"]


def run(inputs, trace=False, **kw):
    nc = _get_program()
    in_maps = _prep_inputs(**inputs)
    res = run_bass_kernel_spmd(nc, in_maps, core_ids=list(range(NCORES)),
                               trace=trace, **kw)
    out = np.concatenate([res.results[i]["out_shard"] for i in range(NCORES)], axis=0)
    return out, res


def kernel(origin_e, U, x, W1, b1, W2, b2, temp):
    out, _ = run(dict(origin_e=origin_e, U=U, x=x, W1=W1, b1=b1, W2=W2,
                      b2=b2, temp=temp))
    return out


# revision 68
# speedup vs baseline: 1.2272x; 1.2272x over previous
"""JacobiGNN Trainium2 kernel: out = log_softmax(U @ (H * (U^T z)), axis=1).

Per-core sharding (core c of 8):
  GEMM1 (row-slab, no z gather): z_c = MLP(x rows) stays local;
    P^T[16,8192] += z_k^T[16,128] @ U[rows_c,:][k][128,8192]   (8 chunks)
    ReduceScatter over spectral -> G_c^T[16,1024].
  GEMM2 (col-slab): out^T[16,8192] += y_sk^T[16,128] @ U^T[cols_c][sk]
    ReduceScatter over nodes -> out rows for core c.

All GEMMs stream U/U^T from DRAM as the bf16 moving operand against tiny
16-wide stationary z/y chunks (1 cycle/row, f32 PSUM accumulation), so the
kernel is HBM-bound at ~32MB/core. out^T/P^T accumulate packed into 4 PSUM
partition-groups (32q offsets). The spectral ReduceScatter hides under the
U^T stream; a dummy sync-engine DMA gates the U^T enqueue so the eviction
descriptors are not buried behind 16MB of queued stream traffic.
"""

import os
import sys

import numpy as np
import ml_dtypes

for _p in ("/opt/trn_rl_repo", "/root/.axon_site/_ro/trn_rl_repo"):
    if os.path.isdir(_p) and _p not in sys.path:
        sys.path.insert(0, _p)

import concourse.bacc as bacc
import concourse.bass as bass  # noqa: F401
import concourse.mybir as mybir
import concourse.tile as tile
from concourse.bass_utils import run_bass_kernel_spmd

F32 = mybir.dt.float32
BF16 = mybir.dt.bfloat16
NPBF16 = ml_dtypes.bfloat16

N, F_IN, HID, C, K = 8192, 512, 64, 16, 10
BASE_ALPHA = 0.5
JA, JB, JL, JR = 1.0, 1.0, -1.0, 1.0
NCORES = 8
SH = N // NCORES          # local spectral columns / node rows (1024)
SK = SH // 128            # local 128-chunks (8)
QN = 4                    # PSUM partition-groups for [16, 8192] packing
QCOLS = N // QN           # cols per group (2048)

_CACHE = {}


def _node_perm():
    """Pack-column -> node for ut_pm. Halves h = pack//4096 are separate
    ReduceScatters; within half h, shard j's slot i maps to outT column
    m = h*512 + i of core j, which after the PE transpose must hold node
    (m%128)*SK + m//128 so the output DMA writes contiguous runs."""
    p = np.arange(N)
    h = p // 4096
    s = p % 4096
    j = s // 512
    m = h * 512 + s % 512
    return j * SH + (m % 128) * SK + m // 128


def _spectral_perm():
    """Pack-column -> spectral for u_row: PSUM quarter q, shard-slot (j, i)
    holds spectral j*1024 + q*256 + i, so each quarter evicts directly as
    one ReduceScatter input and G arrives in 4 pipelined pieces."""
    p = np.arange(N)
    q = p // QCOLS
    j = (p % QCOLS) // 256
    i = p % 256
    return j * SH + q * 256 + i


def _jacobi_coef_rows(temp):
    """Host-precomputed per-channel coefficient rows, [30*C] packed."""
    a, b, l, r = JA, JB, JL, JR
    alphas = (BASE_ALPHA * np.tanh(np.asarray(temp, np.float64)))  # [C, K+1]
    rows = [alphas[:, 0]]
    coef1 = (a - b) / 2 - (a + b + 2) / 2 * (l + r) / (r - l)
    coef2 = (a + b + 2) / (r - l)
    rows.append(coef1 * alphas[:, 1])   # c1_0
    rows.append(coef2 * alphas[:, 1])   # c1_1
    for L in range(2, K + 1):
        coef_l = 2 * L * (L + a + b) * (2 * L - 2 + a + b)
        c_lm1_1 = (2 * L + a + b - 1) * (2 * L + a + b) * (2 * L + a + b - 2)
        c_lm1_2 = (2 * L + a + b - 1) * (a ** 2 - b ** 2)
        c_lm2 = 2 * (L - 1 + a) * (L - 1 + b) * (2 * L + a + b)
        tmp1 = alphas[:, L - 1] * (c_lm1_1 / coef_l)
        tmp2 = alphas[:, L - 1] * (c_lm1_2 / coef_l)
        tmp3 = alphas[:, L - 1] * alphas[:, L - 2] * (c_lm2 / coef_l)
        rows.append(tmp1 * (2 / (r - l)))                    # t1
        rows.append(tmp1 * ((r + l) / (r - l)) + tmp2)       # t2
        rows.append(tmp3)                                    # t3
    packed = np.concatenate(rows).astype(np.float32).reshape(1, 30 * C)
    return np.ascontiguousarray(np.repeat(packed, 128, axis=0))


def _bc(ap, shape, axis=1):
    while ap.ndim < len(shape):
        ap = ap.unsqueeze(axis)
    return ap.broadcast_to(shape)


def _build():
    nc = bacc.Bacc("TRN2", target_bir_lowering=False, debug=False)

    u_row = nc.dram_tensor("u_row", [SH, N], BF16, kind="ExternalInput")
    ut_pm = nc.dram_tensor("ut_pm", [SH, N], BF16, kind="ExternalInput")
    x_sh = nc.dram_tensor("x_shard", [F_IN, SH], BF16, kind="ExternalInput")
    e_sh = nc.dram_tensor("e_shard", [128, SK], F32, kind="ExternalInput")
    w1r = nc.dram_tensor("w1r", [128, 4 * HID], BF16, kind="ExternalInput")
    w2d = nc.dram_tensor("w2d", [HID, C], BF16, kind="ExternalInput")
    b1c = nc.dram_tensor("b1c", [HID, 1], F32, kind="ExternalInput")
    b2c = nc.dram_tensor("b2c", [C, 1], F32, kind="ExternalInput")
    jcd = nc.dram_tensor("jcd", [128, 30 * C], F32, kind="ExternalInput")
    id16d = nc.dram_tensor("id16d", [C, C], F32, kind="ExternalInput")
    out_sh = nc.dram_tensor("out_shard", [SH, C], F32, kind="ExternalOutput")

    rg = [list(range(NCORES))]

    with nc.allow_low_precision(reason="bf16 matmul path"), \
         tile.TileContext(nc) as tc:
        with (
            tc.tile_pool(name="dram", bufs=1, space="DRAM") as dram,
            tc.tile_pool(name="consts", bufs=1) as cp,
            tc.tile_pool(name="persist", bufs=1) as pp,
            tc.tile_pool(name="usb", bufs=16) as up,
            tc.tile_pool(name="small", bufs=4) as sp,
        ):
            rs_g_in = dram.tile([NCORES * C, SH], BF16)
            rs_g_out = dram.tile([C, SH], BF16)
            rs_o_in = [dram.tile([NCORES * C, 512], BF16, name=f"rs_o_in{h}")
                       for h in range(2)]
            rs_o_out = [dram.tile([C, 512], BF16, name=f"rs_o_out{h}")
                        for h in range(2)]
            gate = pp.tile([C, 64], BF16)

            id16 = cp.tile_from(id16d[:])
            jc = cp.tile_from(jcd[:])
            w1 = cp.tile_from(w1r[:])
            w2 = cp.tile_from(w2d[:])
            b1 = cp.tile_from(b1c[:])
            b2 = cp.tile_from(b2c[:])
            e_col = cp.tile_from(e_sh[:])

            zme = pp.tile([128, SK, C], BF16)
            hacc = pp.tile([128, SK, C], F32)
            xs_a = pp.tile([128, SK, C], F32)
            xs_b = pp.tile([128, SK, C], F32)
            htmp = pp.tile([128, SK, C], F32)
            htmp2 = pp.tile([128, SK, C], F32)
            gsbT = pp.tile([C, SH], F32)
            gcol = pp.tile([128, SK, C], F32)
            ybf = pp.tile([128, SK, C], BF16)
            accg = pp.tile([128, QCOLS], BF16)
            acco = pp.tile([128, QCOLS], BF16)
            outT = pp.tile([C, SH], F32)
            smin = pp.tile([128, SK, C], F32)
            smout = pp.tile([128, SK, C], F32)

            # ================= phase 0: MLP head -> z (local only) =======
            with tc.tile_pool(name="mlp", bufs=1) as mp, \
                 tc.tile_pool(name="ppre", bufs=1, space="PSUM") as ppre:
                xT = mp.tile([128, 4, SH], BF16)
                h_sb = mp.tile([HID, SH], BF16)
                zT = mp.tile([C, SH], F32)
                nc.scalar.dma_start(
                    out=xT[:], in_=x_sh[:].rearrange("(a p) r -> p a r", p=128))
                ph = ppre.tile([HID, SH], F32, tag="ph")
                for half in range(2):
                    for fb in range(4):
                        nc.tensor.matmul(
                            ph[:, half * 512:(half + 1) * 512],
                            lhsT=w1[:, fb * HID:(fb + 1) * HID],
                            rhs=xT[:, fb, half * 512:(half + 1) * 512],
                            start=(fb == 0), stop=(fb == 3),
                        )
                nc.scalar.activation(h_sb[:], ph[:], mybir.ActivationFunctionType.Relu,
                                     bias=b1[:, 0:1], scale=1.0)
                pz = ppre.tile([C, SH], F32, tag="pz")
                for half in range(2):
                    nc.tensor.matmul(
                        pz[:, half * 512:(half + 1) * 512],
                        lhsT=w2[:], rhs=h_sb[:, half * 512:(half + 1) * 512],
                        start=True, stop=True,
                    )
                nc.vector.tensor_scalar_add(zT[:], pz[:], b2[:, 0:1])
                # preload Exp/Ln activation tables so the softmax tail does
                # not pay the 1.3us table load
                dex = sp.tile([C, 1], F32, tag="dex")
                nc.scalar.activation(dex[:], b2[:], mybir.ActivationFunctionType.Exp)
                nc.scalar.activation(dex[:], dex[:], mybir.ActivationFunctionType.Ln)
                for rc in range(SK):
                    pzt = ppre.tile([128, C], F32, tag="pzt", bufs=3)
                    nc.tensor.transpose(pzt[:], zT[:, rc * 128:(rc + 1) * 128],
                                        id16[0:C, 0:C])
                    nc.scalar.copy(zme[:, rc, :], pzt[:])

            # utsb opens after the MLP pool closed so it reuses that space
            utp_cm = tc.tile_pool(name="utsb", bufs=28)
            utp = utp_cm.__enter__()

            # ================= Jacobi filter H on DVE ====================
            ev = _bc(e_col[:], (128, SK, C), axis=2)

            def jrow(i):
                return _bc(jc[:, i * C:(i + 1) * C], (128, SK, C))

            nc.vector.tensor_copy(xs_a[:], jrow(0))                       # xs_m2
            nc.vector.tensor_mul(htmp[:], xs_a[:], ev)
            nc.vector.tensor_mul(htmp[:], htmp[:], jrow(2))
            nc.vector.tensor_add(xs_b[:], htmp[:], jrow(1))               # xs_m1
            nc.vector.tensor_add(hacc[:], xs_a[:], xs_b[:])
            xm2, xm1 = xs_a, xs_b
            for L in range(2, K + 1):
                r0 = 3 + 3 * (L - 2)
                nc.vector.tensor_mul(htmp[:], xm1[:], ev)
                nc.vector.tensor_mul(htmp[:], htmp[:], jrow(r0))
                nc.vector.tensor_mul(htmp2[:], xm1[:], jrow(r0 + 1))
                nc.vector.tensor_sub(htmp[:], htmp[:], htmp2[:])
                nc.vector.tensor_mul(htmp2[:], xm2[:], jrow(r0 + 2))
                nc.vector.tensor_sub(xm2[:], htmp[:], htmp2[:])           # nx
                nc.vector.tensor_add(hacc[:], hacc[:], xm2[:])
                xm2, xm1 = xm1, xm2

            # ============ GEMM1: P^T = z^T @ U[rows], then RS-G ==========
            # q-outer: quarter q completes ~17us before quarter q+1, so its
            # eviction copy + shard DMAs ride under the remaining stream and
            # only quarter 3's two shard DMAs precede the RS-G trigger.
            with tc.tile_pool(name="pg", bufs=1, space="PSUM") as pg:
                pacc_g = pg.tile([128, QCOLS], F32, tag="paccg")
                for q in range(QN):
                    for k in range(SK):
                        upc = up.tile([128, QCOLS], BF16, tag="u")
                        nc.sync.dma_start(
                            out=upc[:],
                            in_=u_row[k * 128:(k + 1) * 128,
                                      q * QCOLS:(q + 1) * QCOLS])
                        for off in range(4):
                            nc.tensor.matmul(
                                pacc_g[32 * q:32 * q + C, off * 512:(off + 1) * 512],
                                lhsT=zme[:, k, :], rhs=upc[:, off * 512:(off + 1) * 512],
                                start=(k == 0), stop=(k == SK - 1),
                                skip_group_check=True,
                                tile_position=(0, 32 * q),
                            )
                    ceng = nc.scalar.copy if q % 2 == 0 else nc.vector.tensor_copy
                    ceng(accg[32 * q:32 * q + C, :], pacc_g[32 * q:32 * q + C, :])
                    for j in (2 * q, 2 * q + 1):
                        eng = nc.gpsimd if j % 2 == 0 else nc.scalar
                        eng.dma_start(
                            out=rs_g_in[j * C:(j + 1) * C, :],
                            in_=accg[32 * q:32 * q + C,
                                     (j % 2) * SH:(j % 2 + 1) * SH])
            # gate: the 16MB U^T stream may enqueue only after the RS-G
            # eviction DMAs have landed (keeps them out of queue burial).
            # Must be emitted BEFORE the collective: Tile models the
            # collective as writing rs_g_in, which would serialize the gate
            # (and the whole U^T stream) behind the full RS.
            nc.sync.dma_start(out=gate[:], in_=rs_g_in[0:C, 0:64])
            nc.gpsimd.collective_compute(
                "ReduceScatter", mybir.AluOpType.add, replica_groups=rg,
                ins=[rs_g_in.opt()], outs=[rs_g_out.opt()],
            )

            # y = H * G (transpose G^T back to node-partition layout)
            nc.gpsimd.dma_start(out=gsbT[:], in_=rs_g_out[:])  # bf16 -> f32 cast
            with tc.tile_pool(name="pt", bufs=1, space="PSUM") as pt:
                for sb in range(SK):
                    ptile = pt.tile([128, C], F32, tag="pt", bufs=2)
                    nc.tensor.transpose(ptile[:], gsbT[:, sb * 128:(sb + 1) * 128],
                                        id16[0:C, 0:C])
                    nc.vector.tensor_copy(gcol[:, sb, :], ptile[:])
            nc.vector.tensor_mul(ybf[:], gcol[:], hacc[:])

            # ==== GEMM2: out^T = y^T @ U^T[cols], 2 pipelined RS-out =====
            # q-outer so node-half h completes after quarters 2h, 2h+1; its
            # ReduceScatter + softmax + output DMA hide under half h+1
            sums8 = sp.tile([128, SK], F32, tag="sums")
            lns8 = sp.tile([128, SK], F32, tag="lns")
            et8 = sp.tile([128, SK, C], F32, tag="et")
            with tc.tile_pool(name="pm", bufs=1, space="PSUM") as pm:
                pacc_o = pm.tile([128, QCOLS], F32, tag="pacco")
                for h in range(2):
                    for q in (2 * h, 2 * h + 1):
                        for sk in range(SK):
                            utt = utp.tile([128, QCOLS], BF16, tag="ut")
                            nc.sync.dma_start(
                                out=utt[:],
                                in_=ut_pm[sk * 128:(sk + 1) * 128,
                                          q * QCOLS:(q + 1) * QCOLS])
                            for off in range(4):
                                nc.tensor.matmul(
                                    pacc_o[32 * q:32 * q + C,
                                           off * 512:(off + 1) * 512],
                                    lhsT=ybf[:, sk, :],
                                    rhs=utt[:, off * 512:(off + 1) * 512],
                                    start=(sk == 0), stop=(sk == SK - 1),
                                    skip_group_check=True,
                                    tile_position=(0, 32 * q),
                                )
                    ceng = nc.vector.tensor_copy if h == 0 else nc.scalar.copy
                    ceng(acco[64 * h:64 * h + 48, :], pacc_o[64 * h:64 * h + 48, :])
                    for j in range(NCORES):
                        eng = nc.gpsimd if j % 2 == 0 else nc.scalar
                        eng.dma_start(
                            out=rs_o_in[h][j * C:(j + 1) * C, :],
                            in_=acco[32 * (2 * h + j // 4):32 * (2 * h + j // 4) + C,
                                     (j % 4) * 512:(j % 4 + 1) * 512])
                    nc.gpsimd.collective_compute(
                        "ReduceScatter", mybir.AluOpType.add, replica_groups=rg,
                        ins=[rs_o_in[h].opt()], outs=[rs_o_out[h].opt()],
                    )
                    # local log_softmax + output for node half h
                    nc.gpsimd.dma_start(out=outT[:, h * 512:(h + 1) * 512],
                                        in_=rs_o_out[h][:])  # bf16 -> f32 cast
                    for sb in range(4 * h, 4 * h + 4):
                        ptile = pm.tile([128, C], F32, tag="pt", bufs=2)
                        nc.tensor.transpose(ptile[:], outT[:, sb * 128:(sb + 1) * 128],
                                            id16[0:C, 0:C])
                        nc.scalar.copy(smin[:, sb, :], ptile[:])
                    hs = slice(4 * h, 4 * h + 4)
                    nc.scalar.activation(et8[:, hs, :], smin[:, hs, :],
                                         mybir.ActivationFunctionType.Exp)
                    nc.vector.tensor_reduce(out=sums8[:, hs], in_=et8[:, hs, :],
                                            op=mybir.AluOpType.add,
                                            axis=mybir.AxisListType.X)
                    nc.scalar.activation(lns8[:, hs], sums8[:, hs],
                                         mybir.ActivationFunctionType.Ln)
                    nc.vector.tensor_sub(smout[:, hs, :], smin[:, hs, :],
                                         _bc(lns8[:, hs], (128, 4, C), axis=2))
                    # in-shard node perm makes row r = p*SK + j, so each
                    # partition writes contiguous 256B runs
                    nc.scalar.dma_start(
                        out=out_sh[:].rearrange("(p j) c -> p j c", p=128)[:, hs, :],
                        in_=smout[:, hs, :])
            utp_cm.__exit__(None, None, None)

    nc.compile()
    return nc


def _prep_inputs(origin_e, U, x, W1, b1, W2, b2, temp):
    origin_e = np.asarray(origin_e, np.float32)
    U = np.asarray(U, np.float32)
    x = np.asarray(x, np.float32)
    W1 = np.asarray(W1, np.float32)
    b1 = np.asarray(b1, np.float32)
    W2 = np.asarray(W2, np.float32)
    b2 = np.asarray(b2, np.float32)

    jc = _jacobi_coef_rows(temp)
    id16 = np.eye(C, dtype=np.float32)
    w1r = np.ascontiguousarray(
        W1.reshape(4, 128, HID).transpose(1, 0, 2).reshape(128, 4 * HID)
    ).astype(NPBF16)
    UT = np.ascontiguousarray(U.T[:, _node_perm()])
    shared = {
        "w1r": w1r, "w2d": np.ascontiguousarray(W2).astype(NPBF16),
        "b1c": np.ascontiguousarray(b1.reshape(HID, 1)),
        "b2c": np.ascontiguousarray(b2.reshape(C, 1)),
        "jcd": jc, "id16d": id16,
    }
    in_maps = []
    for i in range(NCORES):
        m = dict(shared)
        m["u_row"] = U[i * SH:(i + 1) * SH, :].astype(NPBF16)
        m["ut_pm"] = UT[i * SH:(i + 1) * SH, :].astype(NPBF16)
        m["x_shard"] = np.ascontiguousarray(x[i * SH:(i + 1) * SH, :].T).astype(NPBF16)
        m["e_shard"] = np.ascontiguousarray(
            origin_e[i * SH:(i + 1) * SH].reshape(SK, 128).T)
        in_maps.append(m)
    return in_maps


def _get_program():
    if "nc" not in _CACHE:
        _CACHE["nc"] = _build()
    return _CACHE["nc"]


def run(inputs, trace=False, **kw):
    nc = _get_program()
    in_maps = _prep_inputs(**inputs)
    res = run_bass_kernel_spmd(nc, in_maps, core_ids=list(range(NCORES)),
                               trace=trace, **kw)
    out = np.concatenate([res.results[i]["out_shard"] for i in range(NCORES)], axis=0)
    return out, res


def kernel(origin_e, U, x, W1, b1, W2, b2, temp):
    out, _ = run(dict(origin_e=origin_e, U=U, x=x, W1=W1, b1=b1, W2=W2,
                      b2=b2, temp=temp))
    return out


# revision 69
# speedup vs baseline: 1.3108x; 1.0681x over previous
"""JacobiGNN Trainium2 kernel: out = log_softmax(U @ (H * (U^T z)), axis=1).

Per-core sharding (core c of 8):
  GEMM1 (row-slab, no z gather): z_c = MLP(x rows) stays local;
    P^T[16,8192] += z_k^T[16,128] @ U[rows_c,:][k][128,8192]   (8 chunks)
    ReduceScatter over spectral -> G_c^T[16,1024].
  GEMM2 (col-slab): out^T[16,8192] += y_sk^T[16,128] @ U^T[cols_c][sk]
    ReduceScatter over nodes -> out rows for core c.

All GEMMs stream U/U^T from DRAM as the bf16 moving operand against tiny
16-wide stationary z/y chunks (1 cycle/row, f32 PSUM accumulation), so the
kernel is HBM-bound at ~32MB/core. out^T/P^T accumulate packed into 4 PSUM
partition-groups (32q offsets). The spectral ReduceScatter hides under the
U^T stream; a dummy sync-engine DMA gates the U^T enqueue so the eviction
descriptors are not buried behind 16MB of queued stream traffic.
"""

import os
import sys

import numpy as np
import ml_dtypes

for _p in ("/opt/trn_rl_repo", "/root/.axon_site/_ro/trn_rl_repo"):
    if os.path.isdir(_p) and _p not in sys.path:
        sys.path.insert(0, _p)

import concourse.bacc as bacc
import concourse.bass as bass  # noqa: F401
import concourse.mybir as mybir
import concourse.tile as tile
from concourse.bass_utils import run_bass_kernel_spmd

F32 = mybir.dt.float32
BF16 = mybir.dt.bfloat16
NPBF16 = ml_dtypes.bfloat16

N, F_IN, HID, C, K = 8192, 512, 64, 16, 10
BASE_ALPHA = 0.5
JA, JB, JL, JR = 1.0, 1.0, -1.0, 1.0
NCORES = 8
SH = N // NCORES          # local spectral columns / node rows (1024)
SK = SH // 128            # local 128-chunks (8)
QN = 4                    # PSUM partition-groups for [16, 8192] packing
QCOLS = N // QN           # cols per group (2048)

_CACHE = {}


def _node_perm():
    """Pack-column -> node for ut_pm. Halves h = pack//4096 are separate
    ReduceScatters; within half h, shard j's slot i maps to outT column
    m = h*512 + i of core j, which after the PE transpose must hold node
    (m%128)*SK + m//128 so the output DMA writes contiguous runs."""
    p = np.arange(N)
    h = p // 4096
    s = p % 4096
    j = s // 512
    m = h * 512 + s % 512
    return j * SH + (m % 128) * SK + m // 128


def _spectral_perm():
    """Pack-column -> spectral for u_row: PSUM quarter q, shard-slot (j, i)
    holds spectral j*1024 + q*256 + i, so each quarter evicts directly as
    one ReduceScatter input and G arrives in 4 pipelined pieces."""
    p = np.arange(N)
    q = p // QCOLS
    j = (p % QCOLS) // 256
    i = p % 256
    return j * SH + q * 256 + i


def _jacobi_coef_rows(temp):
    """Host-precomputed per-channel coefficient rows, [30*C] packed."""
    a, b, l, r = JA, JB, JL, JR
    alphas = (BASE_ALPHA * np.tanh(np.asarray(temp, np.float64)))  # [C, K+1]
    rows = [alphas[:, 0]]
    coef1 = (a - b) / 2 - (a + b + 2) / 2 * (l + r) / (r - l)
    coef2 = (a + b + 2) / (r - l)
    rows.append(coef1 * alphas[:, 1])   # c1_0
    rows.append(coef2 * alphas[:, 1])   # c1_1
    for L in range(2, K + 1):
        coef_l = 2 * L * (L + a + b) * (2 * L - 2 + a + b)
        c_lm1_1 = (2 * L + a + b - 1) * (2 * L + a + b) * (2 * L + a + b - 2)
        c_lm1_2 = (2 * L + a + b - 1) * (a ** 2 - b ** 2)
        c_lm2 = 2 * (L - 1 + a) * (L - 1 + b) * (2 * L + a + b)
        tmp1 = alphas[:, L - 1] * (c_lm1_1 / coef_l)
        tmp2 = alphas[:, L - 1] * (c_lm1_2 / coef_l)
        tmp3 = alphas[:, L - 1] * alphas[:, L - 2] * (c_lm2 / coef_l)
        rows.append(tmp1 * (2 / (r - l)))                    # t1
        rows.append(tmp1 * ((r + l) / (r - l)) + tmp2)       # t2
        rows.append(tmp3)                                    # t3
    packed = np.concatenate(rows).astype(np.float32).reshape(1, 30 * C)
    return np.ascontiguousarray(np.repeat(packed, 128, axis=0))


def _bc(ap, shape, axis=1):
    while ap.ndim < len(shape):
        ap = ap.unsqueeze(axis)
    return ap.broadcast_to(shape)


def _build():
    nc = bacc.Bacc("TRN2", target_bir_lowering=False, debug=False)

    u_row = nc.dram_tensor("u_row", [SH, N], BF16, kind="ExternalInput")
    ut_pm = nc.dram_tensor("ut_pm", [SH, N], BF16, kind="ExternalInput")
    x_sh = nc.dram_tensor("x_shard", [F_IN, SH], BF16, kind="ExternalInput")
    e_sh = nc.dram_tensor("e_shard", [128, SK], F32, kind="ExternalInput")
    w1r = nc.dram_tensor("w1r", [128, 4 * HID], BF16, kind="ExternalInput")
    w2d = nc.dram_tensor("w2d", [HID, C], BF16, kind="ExternalInput")
    b1c = nc.dram_tensor("b1c", [HID, 1], F32, kind="ExternalInput")
    b2c = nc.dram_tensor("b2c", [C, 1], F32, kind="ExternalInput")
    jcd = nc.dram_tensor("jcd", [128, 30 * C], F32, kind="ExternalInput")
    id16d = nc.dram_tensor("id16d", [C, C], F32, kind="ExternalInput")
    out_sh = nc.dram_tensor("out_shard", [SH, C], F32, kind="ExternalOutput")

    rg = [list(range(NCORES))]

    with nc.allow_low_precision(reason="bf16 matmul path"), \
         tile.TileContext(nc) as tc:
        with (
            tc.tile_pool(name="dram", bufs=1, space="DRAM") as dram,
            tc.tile_pool(name="consts", bufs=1) as cp,
            tc.tile_pool(name="persist", bufs=1) as pp,
            tc.tile_pool(name="usb", bufs=16) as up,
            tc.tile_pool(name="small", bufs=4) as sp,
        ):
            rs_g_in = dram.tile([NCORES * C, SH], BF16)
            rs_g_out = dram.tile([C, SH], BF16)
            rs_o_in = [dram.tile([NCORES * C, 512], BF16, name=f"rs_o_in{h}")
                       for h in range(2)]
            rs_o_out = [dram.tile([C, 512], BF16, name=f"rs_o_out{h}")
                        for h in range(2)]
            gate = pp.tile([C, 64], BF16)

            id16 = cp.tile_from(id16d[:])
            jc = cp.tile_from(jcd[:])
            w1 = cp.tile_from(w1r[:])
            w2 = cp.tile_from(w2d[:])
            b1 = cp.tile_from(b1c[:])
            b2 = cp.tile_from(b2c[:])
            e_col = cp.tile_from(e_sh[:])

            zme = pp.tile([128, SK, C], BF16)
            hacc = pp.tile([128, SK, C], F32)
            xs_a = pp.tile([128, SK, C], F32)
            xs_b = pp.tile([128, SK, C], F32)
            htmp = pp.tile([128, SK, C], F32)
            htmp2 = pp.tile([128, SK, C], F32)
            gsbT = pp.tile([C, SH], F32)
            gcol = pp.tile([128, SK, C], F32)
            ybf = pp.tile([128, SK, C], BF16)
            accg = pp.tile([128, QCOLS], BF16)
            acco = pp.tile([128, QCOLS], BF16)
            outT = pp.tile([C, SH], F32)
            smin = pp.tile([128, SK, C], F32)
            smout = pp.tile([128, SK, C], F32)

            # ================= phase 0: MLP head -> z (local only) =======
            with tc.tile_pool(name="mlp", bufs=1) as mp, \
                 tc.tile_pool(name="ppre", bufs=1, space="PSUM") as ppre:
                xT = mp.tile([128, 4, SH], BF16)
                h_sb = mp.tile([HID, SH], BF16)
                zT = mp.tile([C, SH], F32)
                nc.scalar.dma_start(
                    out=xT[:], in_=x_sh[:].rearrange("(a p) r -> p a r", p=128))
                ph = ppre.tile([HID, SH], F32, tag="ph")
                for half in range(2):
                    for fb in range(4):
                        nc.tensor.matmul(
                            ph[:, half * 512:(half + 1) * 512],
                            lhsT=w1[:, fb * HID:(fb + 1) * HID],
                            rhs=xT[:, fb, half * 512:(half + 1) * 512],
                            start=(fb == 0), stop=(fb == 3),
                        )
                nc.scalar.activation(h_sb[:], ph[:], mybir.ActivationFunctionType.Relu,
                                     bias=b1[:, 0:1], scale=1.0)
                pz = ppre.tile([C, SH], F32, tag="pz")
                for half in range(2):
                    nc.tensor.matmul(
                        pz[:, half * 512:(half + 1) * 512],
                        lhsT=w2[:], rhs=h_sb[:, half * 512:(half + 1) * 512],
                        start=True, stop=True,
                    )
                nc.vector.tensor_scalar_add(zT[:], pz[:], b2[:, 0:1])
                # preload Exp/Ln activation tables so the softmax tail does
                # not pay the 1.3us table load
                dex = sp.tile([C, 1], F32, tag="dex")
                nc.scalar.activation(dex[:], b2[:], mybir.ActivationFunctionType.Exp)
                nc.scalar.activation(dex[:], dex[:], mybir.ActivationFunctionType.Ln)
                for rc in range(SK):
                    pzt = ppre.tile([128, C], F32, tag="pzt", bufs=3)
                    nc.tensor.transpose(pzt[:], zT[:, rc * 128:(rc + 1) * 128],
                                        id16[0:C, 0:C])
                    nc.scalar.copy(zme[:, rc, :], pzt[:])

            # utsb opens after the MLP pool closed so it reuses that space
            utp_cm = tc.tile_pool(name="utsb", bufs=28)
            utp = utp_cm.__enter__()

            # ================= Jacobi filter H on DVE ====================
            ev = _bc(e_col[:], (128, SK, C), axis=2)

            def jrow(i):
                return _bc(jc[:, i * C:(i + 1) * C], (128, SK, C))

            nc.vector.tensor_copy(xs_a[:], jrow(0))                       # xs_m2
            nc.vector.tensor_mul(htmp[:], xs_a[:], ev)
            nc.vector.tensor_mul(htmp[:], htmp[:], jrow(2))
            nc.vector.tensor_add(xs_b[:], htmp[:], jrow(1))               # xs_m1
            nc.vector.tensor_add(hacc[:], xs_a[:], xs_b[:])
            xm2, xm1 = xs_a, xs_b
            for L in range(2, K + 1):
                r0 = 3 + 3 * (L - 2)
                nc.vector.tensor_mul(htmp[:], xm1[:], ev)
                nc.vector.tensor_mul(htmp[:], htmp[:], jrow(r0))
                nc.vector.tensor_mul(htmp2[:], xm1[:], jrow(r0 + 1))
                nc.vector.tensor_sub(htmp[:], htmp[:], htmp2[:])
                nc.vector.tensor_mul(htmp2[:], xm2[:], jrow(r0 + 2))
                nc.vector.tensor_sub(xm2[:], htmp[:], htmp2[:])           # nx
                nc.vector.tensor_add(hacc[:], hacc[:], xm2[:])
                xm2, xm1 = xm1, xm2

            # ============ GEMM1: P^T = z^T @ U[rows], then RS-G ==========
            # q-outer: quarter q completes ~17us before quarter q+1, so its
            # eviction copy + shard DMAs ride under the remaining stream and
            # only quarter 3's two shard DMAs precede the RS-G trigger.
            with tc.tile_pool(name="pg", bufs=1, space="PSUM") as pg:
                pacc_g = pg.tile([128, QCOLS], F32, tag="paccg")
                for q in range(QN):
                    for k in range(SK):
                        upc = up.tile([128, QCOLS], BF16, tag="u")
                        nc.sync.dma_start(
                            out=upc[:],
                            in_=u_row[k * 128:(k + 1) * 128,
                                      q * QCOLS:(q + 1) * QCOLS])
                        for off in range(4):
                            nc.tensor.matmul(
                                pacc_g[32 * q:32 * q + C, off * 512:(off + 1) * 512],
                                lhsT=zme[:, k, :], rhs=upc[:, off * 512:(off + 1) * 512],
                                start=(k == 0), stop=(k == SK - 1),
                                skip_group_check=True,
                                tile_position=(0, 32 * q),
                            )
                    ceng = nc.scalar.copy if q % 2 == 0 else nc.vector.tensor_copy
                    ceng(accg[32 * q:32 * q + C, :], pacc_g[32 * q:32 * q + C, :])
                    for j in (2 * q, 2 * q + 1):
                        eng = nc.gpsimd if j % 2 == 0 else nc.scalar
                        eng.dma_start(
                            out=rs_g_in[j * C:(j + 1) * C, :],
                            in_=accg[32 * q:32 * q + C,
                                     (j % 2) * SH:(j % 2 + 1) * SH])
            # gate: the 16MB U^T stream may enqueue only after the RS-G
            # eviction DMAs have landed (keeps them out of queue burial).
            # Must be emitted BEFORE the collective: Tile models the
            # collective as writing rs_g_in, which would serialize the gate
            # (and the whole U^T stream) behind the full RS.
            nc.sync.dma_start(out=gate[:], in_=rs_g_in[0:C, 0:64])
            nc.gpsimd.collective_compute(
                "ReduceScatter", mybir.AluOpType.add, replica_groups=rg,
                ins=[rs_g_in.opt()], outs=[rs_g_out.opt()],
            )

            # y = H * G (transpose G^T back to node-partition layout)
            nc.gpsimd.dma_start(out=gsbT[:], in_=rs_g_out[:])  # bf16 -> f32 cast
            with tc.tile_pool(name="pt", bufs=1, space="PSUM") as pt:
                for sb in range(SK):
                    ptile = pt.tile([128, C], F32, tag="pt", bufs=2)
                    nc.tensor.transpose(ptile[:], gsbT[:, sb * 128:(sb + 1) * 128],
                                        id16[0:C, 0:C])
                    nc.vector.tensor_copy(gcol[:, sb, :], ptile[:])
            nc.vector.tensor_mul(ybf[:], gcol[:], hacc[:])

            # ==== GEMM2: out^T = y^T @ U^T[cols], 2 pipelined RS-out =====
            # q-outer so node-half h completes after quarters 2h, 2h+1; its
            # ReduceScatter + softmax + output DMA hide under half h+1
            sums8 = sp.tile([128, SK], F32, tag="sums")
            lns8 = sp.tile([128, SK], F32, tag="lns")
            et8 = sp.tile([128, SK, C], F32, tag="et")
            with tc.tile_pool(name="pm", bufs=1, space="PSUM") as pm:
                pacc_o = pm.tile([128, QCOLS], F32, tag="pacco")
                n_ut = 0
                for h in range(2):
                    for q in (2 * h, 2 * h + 1):
                        for sk in range(SK):
                            utt = utp.tile([128, QCOLS], BF16, tag="ut")
                            n_ut += 1
                            if n_ut == 13:
                                # tranche 2 of the U^T stream waits for RS-G
                                # to finish, so the mesh only contends with
                                # 6MB of queued stream instead of 14MB
                                nc.sync.dma_start(out=gate[:],
                                                 in_=rs_g_out[0:C, 0:64])
                            nc.sync.dma_start(
                                out=utt[:],
                                in_=ut_pm[sk * 128:(sk + 1) * 128,
                                          q * QCOLS:(q + 1) * QCOLS])
                            for off in range(4):
                                nc.tensor.matmul(
                                    pacc_o[32 * q:32 * q + C,
                                           off * 512:(off + 1) * 512],
                                    lhsT=ybf[:, sk, :],
                                    rhs=utt[:, off * 512:(off + 1) * 512],
                                    start=(sk == 0), stop=(sk == SK - 1),
                                    skip_group_check=True,
                                    tile_position=(0, 32 * q),
                                )
                    ceng = nc.vector.tensor_copy if h == 0 else nc.scalar.copy
                    ceng(acco[64 * h:64 * h + 48, :], pacc_o[64 * h:64 * h + 48, :])
                    for j in range(NCORES):
                        eng = nc.gpsimd if j % 2 == 0 else nc.scalar
                        eng.dma_start(
                            out=rs_o_in[h][j * C:(j + 1) * C, :],
                            in_=acco[32 * (2 * h + j // 4):32 * (2 * h + j // 4) + C,
                                     (j % 4) * 512:(j % 4 + 1) * 512])
                    nc.gpsimd.collective_compute(
                        "ReduceScatter", mybir.AluOpType.add, replica_groups=rg,
                        ins=[rs_o_in[h].opt()], outs=[rs_o_out[h].opt()],
                    )
                    # local log_softmax + output for node half h
                    nc.gpsimd.dma_start(out=outT[:, h * 512:(h + 1) * 512],
                                        in_=rs_o_out[h][:])  # bf16 -> f32 cast
                    for sb in range(4 * h, 4 * h + 4):
                        ptile = pm.tile([128, C], F32, tag="pt", bufs=2)
                        nc.tensor.transpose(ptile[:], outT[:, sb * 128:(sb + 1) * 128],
                                            id16[0:C, 0:C])
                        nc.scalar.copy(smin[:, sb, :], ptile[:])
                    hs = slice(4 * h, 4 * h + 4)
                    nc.scalar.activation(et8[:, hs, :], smin[:, hs, :],
                                         mybir.ActivationFunctionType.Exp)
                    nc.vector.tensor_reduce(out=sums8[:, hs], in_=et8[:, hs, :],
                                            op=mybir.AluOpType.add,
                                            axis=mybir.AxisListType.X)
                    nc.scalar.activation(lns8[:, hs], sums8[:, hs],
                                         mybir.ActivationFunctionType.Ln)
                    nc.vector.tensor_sub(smout[:, hs, :], smin[:, hs, :],
                                         _bc(lns8[:, hs], (128, 4, C), axis=2))
                    # in-shard node perm makes row r = p*SK + j, so each
                    # partition writes contiguous 256B runs
                    nc.scalar.dma_start(
                        out=out_sh[:].rearrange("(p j) c -> p j c", p=128)[:, hs, :],
                        in_=smout[:, hs, :])
            utp_cm.__exit__(None, None, None)

    nc.compile()
    return nc


def _prep_inputs(origin_e, U, x, W1, b1, W2, b2, temp):
    origin_e = np.asarray(origin_e, np.float32)
    U = np.asarray(U, np.float32)
    x = np.asarray(x, np.float32)
    W1 = np.asarray(W1, np.float32)
    b1 = np.asarray(b1, np.float32)
    W2 = np.asarray(W2, np.float32)
    b2 = np.asarray(b2, np.float32)

    jc = _jacobi_coef_rows(temp)
    id16 = np.eye(C, dtype=np.float32)
    w1r = np.ascontiguousarray(
        W1.reshape(4, 128, HID).transpose(1, 0, 2).reshape(128, 4 * HID)
    ).astype(NPBF16)
    UT = np.ascontiguousarray(U.T[:, _node_perm()])
    shared = {
        "w1r": w1r, "w2d": np.ascontiguousarray(W2).astype(NPBF16),
        "b1c": np.ascontiguousarray(b1.reshape(HID, 1)),
        "b2c": np.ascontiguousarray(b2.reshape(C, 1)),
        "jcd": jc, "id16d": id16,
    }
    in_maps = []
    for i in range(NCORES):
        m = dict(shared)
        m["u_row"] = U[i * SH:(i + 1) * SH, :].astype(NPBF16)
        m["ut_pm"] = UT[i * SH:(i + 1) * SH, :].astype(NPBF16)
        m["x_shard"] = np.ascontiguousarray(x[i * SH:(i + 1) * SH, :].T).astype(NPBF16)
        m["e_shard"] = np.ascontiguousarray(
            origin_e[i * SH:(i + 1) * SH].reshape(SK, 128).T)
        in_maps.append(m)
    return in_maps


def _get_program():
    if "nc" not in _CACHE:
        _CACHE["nc"] = _build()
    return _CACHE["nc"]


def run(inputs, trace=False, **kw):
    nc = _get_program()
    in_maps = _prep_inputs(**inputs)
    res = run_bass_kernel_spmd(nc, in_maps, core_ids=list(range(NCORES)),
                               trace=trace, **kw)
    out = np.concatenate([res.results[i]["out_shard"] for i in range(NCORES)], axis=0)
    return out, res


def kernel(origin_e, U, x, W1, b1, W2, b2, temp):
    out, _ = run(dict(origin_e=origin_e, U=U, x=x, W1=W1, b1=b1, W2=W2,
                      b2=b2, temp=temp))
    return out
